# revision 11
# baseline (speedup 1.0000x reference)
"""BOW classifier kernel for 8 Trainium2 NeuronCores.

Counts-matmul formulation with 4-way vocab x 2-way batch sharding.
The masked mean-pool
  pooled[b] = (1/len[b]) * sum_{s<len[b]} emb[text[s,b]]
is a sparse matmul  pooled = counts @ emb  with counts[b,v] the number
of times token v appears in the first len[b] positions of column b (the
1/len is folded into counts on the host).  Core i = (half hb=i//4,
quarter q=i%4) owns vocab rows [q*12544, (q+1)*12544) of the padded
bf16 table and batch columns [512*hb, 512*hb+512) of scaled counts^T,
computes its partial pooled on the tensor engine (bf16 x bf16 -> fp32
PSUM), and a 4-core ReduceScatter (groups [0..3] and [4..7], running
concurrently) sums the vocab quarters and hands core i batch rows
[128*i, 128*(i+1)).  The MLP tail (fc1 bias fold + relu + fc2, bf16
inputs with fp32 PSUM accumulate) runs per-core on its 128 batch rows.

The 4-core group halves the collective payload (307 KB bf16) and drops
an RDH round vs the 8-core variant.  Dummy matmuls on memset tiles warm
the PE pstate during the initial DMA fill; counts DMAs issue from the
sync engine two chunks at a time and embedding/weight DMAs from gpsimd
so the issue rate stays ahead of the PE.
"""

import sys

import numpy as np

for _p in ("/opt/trn_rl_repo",):
    if _p not in sys.path:
        sys.path.insert(0, _p)

V, E, H, O = 50000, 300, 512, 2
S, B = 512, 1024
NCORES = 8
VQ = 4              # vocab quarters
BH = 2              # batch halves
VSH = 12544         # padded vocab rows per core (98 * 128)
VP = VQ * VSH       # 50176 padded vocab rows total
KC = VSH // 128     # 98 contraction chunks per core
BSH = B // BH       # 512 batch columns per core
BG = BSH // 128     # 4 batch groups of 128
BS = B // NCORES    # 128 batch rows per core after reduce-scatter
NWARM = 30          # dummy matmuls to ramp the PE pstate


def _build_nc(repeat=None):
    import os
    from contextlib import ExitStack

    if repeat is None:
        repeat = int(os.environ.get("KERNEL_REPEAT", "1"))

    import concourse.tile as tile
    from concourse import bacc, bass, mybir
    from concourse.masks import make_identity

    bf16, f32 = mybir.dt.bfloat16, mybir.dt.float32

    nc = bacc.Bacc(None, target_bir_lowering=False, num_devices=NCORES)
    cnt_d = nc.declare_dram_parameter("cnt", [VSH, BSH], bf16, isOutput=False)
    emb_d = nc.declare_dram_parameter("emb", [VSH, E], bf16, isOutput=False)
    w1b_d = nc.declare_dram_parameter("w1b", [E + 1, H], bf16, isOutput=False)
    w2b_d = nc.declare_dram_parameter("w2b", [H + 1, O], bf16, isOutput=False)
    out_d = nc.declare_dram_parameter("out", [BS, O], f32, isOutput=True)

    with tile.TileContext(nc) as tc, ExitStack() as ctx:
        sb = ctx.enter_context(tc.tile_pool(name="sb", bufs=1))
        dram = ctx.enter_context(tc.tile_pool(name="dram", bufs=1, space="DRAM"))

        # counts: two 128-row chunks per DMA to keep the sync-engine issue
        # rate (~0.7 us/instr) ahead of the PE consume rate
        cnt_t = []
        for j in range(KC // 2):
            ct = sb.tile([128, 2 * BSH], bf16, tag=f"cnt{j}", name=f"cnt{j}")
            nc.sync.dma_start(
                out=ct[:].rearrange("p (t c) -> p t c", t=2),
                in_=cnt_d[j * 256:(j + 1) * 256, :].rearrange(
                    "(t p) c -> p t c", t=2),
            )
            cnt_t.append(ct)
        emb_t = []
        for j in range(KC // 2):
            et = sb.tile([128, 2 * E], bf16, tag=f"emb{j}", name=f"emb{j}")
            nc.gpsimd.dma_start(
                out=et[:].rearrange("p (t c) -> p t c", t=2),
                in_=emb_d[j * 256:(j + 1) * 256, :].rearrange(
                    "(t p) c -> p t c", t=2),
            )
            emb_t.append(et)

        w1_t = []
        for c, (r0, r1) in enumerate([(0, 128), (128, 256), (256, E + 1)]):
            t = sb.tile([r1 - r0, H], bf16, tag=f"w1_{c}", name=f"w1_{c}")
            nc.gpsimd.dma_start(out=t[:], in_=w1b_d[r0:r1, :])
            w1_t.append(t)
        w2_t = []
        for c in range(4):
            t = sb.tile([128, O], bf16, tag=f"w2_{c}", name=f"w2_{c}")
            nc.gpsimd.dma_start(out=t[:], in_=w2b_d[c * 128:(c + 1) * 128, :])
            w2_t.append(t)
        b2_t = sb.tile([1, O], bf16, tag="b2")
        nc.gpsimd.dma_start(out=b2_t[:], in_=w2b_d[H:H + 1, :])

        # PE pstate warm-up on memset tiles (no DMA dependency); the real
        # accumulation below opens with start=True, discarding this junk
        wa = sb.tile([128, 128], bf16, tag="wa")
        nc.vector.memset(wa[:], 0.0)
        wb = sb.tile([128, E], bf16, tag="wb")
        nc.vector.memset(wb[:], 0.0)

        pooled_all = sb.tile([128, BG * E], bf16, tag="pooled_all")
        with tc.tile_pool(name="psA", bufs=1, space="PSUM") as psA:
            acc = [
                psA.tile([128, 512], f32, tag=f"acc{g}", name=f"acc{g}")
                for g in range(BG)
            ]
            for w in range(NWARM):
                nc.tensor.matmul(out=acc[0][:, 0:E], lhsT=wa[:], rhs=wb[:],
                                 start=True, stop=True)
            for rep in range(repeat):
                for k in range(KC):
                    ct = cnt_t[k // 2]
                    et = emb_t[k // 2]
                    t = k % 2
                    for g in range(BG):
                        nc.tensor.matmul(
                            out=acc[g][:, 0:E],
                            lhsT=ct[:, t * BSH + g * 128:t * BSH + (g + 1) * 128],
                            rhs=et[:, t * E:(t + 1) * E],
                            start=(k == 0),
                            stop=(k == KC - 1),
                        )
            # drain the accumulators (pipelines behind the last matmuls;
            # gpsimd cannot read PSUM)
            for g in range(BG):
                nc.vector.tensor_copy(
                    out=pooled_all[:, g * E:(g + 1) * E], in_=acc[g][:, 0:E]
                )

        # 4-core sum over vocab quarters + scatter: cores [4hb..4hb+3]
        # exchange batch half hb; core i keeps batch rows [128i, 128i+128)
        part_d = dram.tile([BSH, E], bf16)
        rs_d = dram.tile([BS, E], bf16)
        nc.gpsimd.dma_start(
            out=part_d[:].rearrange("(g p) e -> p g e", g=BG),
            in_=pooled_all[:].rearrange("p (g e) -> p g e", g=BG),
        )
        nc.gpsimd.collective_compute(
            "ReduceScatter",
            mybir.AluOpType.add,
            replica_groups=[[0, 1, 2, 3], [4, 5, 6, 7]],
            ins=[part_d.opt()],
            outs=[rs_d.opt()],
        )
        pooled = sb.tile([BS, E], bf16, tag="pooled")
        nc.gpsimd.dma_start(out=pooled[:], in_=rs_d[:])

        with tc.tile_pool(name="ps", bufs=1, space="PSUM") as ps, \
                tc.tile_pool(name="ps2", bufs=2, space="PSUM") as ps2:
            # fc1: h = relu(pooled @ W1 + b1), contraction via pooled^T on PE
            ident = sb.tile([128, 128], bf16, tag="ident")
            make_identity(nc, ident[:])
            lhs = []
            for c, (c0, c1) in enumerate([(0, 128), (128, 256), (256, E)]):
                w = c1 - c0
                pt = ps2.tile([w, 128], bf16, tag="tr", space="PSUM")
                nc.tensor.transpose(out=pt[:], in_=pooled[:, c0:c1],
                                    identity=ident[:])
                rows = w + 1 if c == 2 else w
                lt = sb.tile([rows, 128], bf16, tag=f"lhs{c}", name=f"lhs{c}")
                if c == 2:
                    # row `w` must be ones (bias row); memset whole tile first
                    # (partition-offset writes must start at partition 0)
                    nc.vector.memset(lt[:], 1.0)
                nc.vector.tensor_copy(out=lt[0:w, :], in_=pt[:])
                lhs.append(lt)
            hp = ps.tile([128, H], f32, tag="hp", space="PSUM")
            for c in range(3):
                nc.tensor.matmul(
                    out=hp[:], lhsT=lhs[c][:], rhs=w1_t[c][:],
                    start=(c == 0), stop=(c == 2),
                )
            h = sb.tile([128, H], bf16, tag="h")
            nc.scalar.activation(out=h[:], in_=hp[:],
                                 func=mybir.ActivationFunctionType.Relu)

            # fc2: out = h @ W2 + b2
            ones1 = sb.tile([1, 128], bf16, tag="ones1")
            nc.vector.memset(ones1[:], 1.0)
            op_ = ps.tile([128, O], f32, tag="op", space="PSUM")
            for c in range(4):
                pt = ps2.tile([128, 128], bf16, tag="tr2", space="PSUM")
                nc.tensor.transpose(out=pt[:], in_=h[:, c * 128:(c + 1) * 128],
                                    identity=ident[:])
                ht = sb.tile([128, 128], bf16, tag=f"ht{c}", name=f"ht{c}")
                nc.vector.tensor_copy(out=ht[:], in_=pt[:])
                nc.tensor.matmul(out=op_[:], lhsT=ht[:], rhs=w2_t[c][:],
                                 start=(c == 0), stop=False)
            nc.tensor.matmul(out=op_[:], lhsT=ones1[:], rhs=b2_t[:],
                             start=False, stop=True)
            out_sb = sb.tile([BS, O], f32, tag="osb")
            nc.vector.tensor_copy(out=out_sb[:], in_=op_[:])
            nc.sync.dma_start(out=out_d[:], in_=out_sb[:])

    nc.finalize()
    return nc


def _prep_in_maps(text, lengths, emb_table, W1, b1, W2, b2):
    import ml_dtypes

    bf16 = ml_dtypes.bfloat16
    text = np.asarray(text, dtype=np.int64)         # [S, B]
    lengths = np.asarray(lengths, dtype=np.int64)   # [B]

    # counts^T [VP, B] scaled by 1/len: row v = per-batch frequency of
    # token v among the first len[b] positions (vocab-major for sharding)
    mask = np.arange(S)[:, None] < lengths[None, :]
    flat = (text * B + np.arange(B)[None, :])[mask]
    cntT = np.bincount(flat, minlength=VP * B).reshape(VP, B)
    inv_len = (1.0 / lengths.astype(np.float32)).astype(np.float32)
    cntT16 = (cntT * inv_len[None, :]).astype(bf16)

    embp = np.zeros((VP, E), np.float32)
    embp[:V] = np.asarray(emb_table, np.float32)
    emb16 = embp.astype(bf16)

    w1b = np.vstack([np.asarray(W1, np.float32),
                     np.asarray(b1, np.float32)[None, :]]).astype(bf16)
    w2b = np.vstack([np.asarray(W2, np.float32),
                     np.asarray(b2, np.float32)[None, :]]).astype(bf16)

    in_maps = []
    for i in range(NCORES):
        hb, q = i // VQ, i % VQ
        in_maps.append({
            "cnt": np.ascontiguousarray(
                cntT16[q * VSH:(q + 1) * VSH, hb * BSH:(hb + 1) * BSH]),
            "emb": np.ascontiguousarray(emb16[q * VSH:(q + 1) * VSH]),
            "w1b": w1b,
            "w2b": w2b,
        })
    return in_maps


def _run(inputs, trace=False):
    from concourse.bass_utils import run_bass_kernel_spmd

    nc = _build_nc()
    in_maps = _prep_in_maps(**inputs)
    res = run_bass_kernel_spmd(nc, in_maps, list(range(NCORES)), trace=trace)
    out = np.concatenate([res.results[i]["out"] for i in range(NCORES)], axis=0)
    return out.astype(np.float32), res


def kernel(**inputs):
    out, _ = _run(inputs, trace=False)
    return out


# revision 12
# speedup vs baseline: 1.2348x; 1.2348x over previous
"""BOW classifier kernel for 8 Trainium2 NeuronCores.

Vocab-sharded counts-matmul formulation.  The masked mean-pool
  pooled[b] = (1/len[b]) * sum_{s<len[b]} emb[text[s,b]]
is a sparse matmul  pooled = counts @ emb  with counts[b,v] the number of
times token v appears in the first len[b] positions of column b (the
1/len is folded into counts on the host).  Each core owns a 6272-row
slice of the (padded, bf16) embedding table and the matching slice of
counts^T, computes its partial pooled on the tensor engine (bf16 x bf16
-> fp32 PSUM), and a bf16 ReduceScatter sums the partials and hands core
i batch rows [128*i, 128*(i+1)).  The MLP tail (fc1 bias fold + relu +
fc2, bf16 inputs with fp32 PSUM accumulate) runs per-core on its 128
batch rows.

Schedule notes: dummy matmuls on memset tiles ramp the PE pstate during
the initial DMA fill (the real accumulation opens with start=True, so
the junk is discarded); counts DMAs issue two 128-row chunks per
instruction from the sync engine while embedding DMAs go through the
scalar-engine HWDGE and weight DMAs through gpsimd, giving three
parallel issue paths so the PE (~1.0 us/chunk consume rate) never
starves.  The ReduceScatter is triggered as soon as the accumulator
drains land; its start is pinned by NRT's fixed first-collective
barrier (~70 us), which the matmul phase hides.
"""

import sys

import numpy as np

for _p in ("/opt/trn_rl_repo",):
    if _p not in sys.path:
        sys.path.insert(0, _p)

V, E, H, O = 50000, 300, 512, 2
S, B = 512, 1024
NCORES = 8
VSH = 6272          # padded vocab rows per core (49 * 128)
VP = NCORES * VSH   # 50176 padded vocab rows total
KC = VSH // 128     # 49 contraction chunks per core
BG = B // 128       # 8 batch groups of 128
BS = B // NCORES    # 128 batch rows per core after reduce-scatter
NWARM = 30          # dummy matmuls to ramp the PE pstate


def _build_nc(repeat=None):
    import os
    from contextlib import ExitStack

    if repeat is None:
        repeat = int(os.environ.get("KERNEL_REPEAT", "1"))

    import concourse.tile as tile
    from concourse import bacc, bass, mybir
    from concourse.masks import make_identity

    bf16, f32 = mybir.dt.bfloat16, mybir.dt.float32

    nc = bacc.Bacc(None, target_bir_lowering=False, num_devices=NCORES)
    cnt_d = nc.declare_dram_parameter("cnt", [VSH, B], bf16, isOutput=False)
    emb_d = nc.declare_dram_parameter("emb", [VSH, E], bf16, isOutput=False)
    w1b_d = nc.declare_dram_parameter("w1b", [E + 1, H], bf16, isOutput=False)
    w2b_d = nc.declare_dram_parameter("w2b", [H + 1, O], bf16, isOutput=False)
    out_d = nc.declare_dram_parameter("out", [BS, O], f32, isOutput=True)

    with tile.TileContext(nc) as tc, ExitStack() as ctx:
        sb = ctx.enter_context(tc.tile_pool(name="sb", bufs=1))
        dram = ctx.enter_context(tc.tile_pool(name="dram", bufs=1, space="DRAM"))

        # counts: two 128-row chunks per DMA (sync engine); embeddings on
        # the scalar-engine HWDGE; weights on gpsimd
        cnt_t = []
        for j in range((KC + 1) // 2):
            r1 = min((j + 1) * 256, VSH)
            t2 = (r1 - j * 256) // 128
            ct = sb.tile([128, t2 * B], bf16, tag=f"cnt{j}", name=f"cnt{j}")
            nc.sync.dma_start(
                out=ct[:].rearrange("p (t c) -> p t c", t=t2),
                in_=cnt_d[j * 256:r1, :].rearrange("(t p) c -> p t c", t=t2),
            )
            cnt_t.append(ct)
        emb_t = []
        for k in range(KC):
            et = sb.tile([128, E], bf16, tag=f"emb{k}", name=f"emb{k}")
            nc.scalar.dma_start(out=et[:], in_=emb_d[k * 128:(k + 1) * 128, :])
            emb_t.append(et)

        w1_t = []
        for c, (r0, r1) in enumerate([(0, 128), (128, 256), (256, E + 1)]):
            t = sb.tile([r1 - r0, H], bf16, tag=f"w1_{c}", name=f"w1_{c}")
            nc.gpsimd.dma_start(out=t[:], in_=w1b_d[r0:r1, :])
            w1_t.append(t)
        w2_t = []
        for c in range(4):
            t = sb.tile([128, O], bf16, tag=f"w2_{c}", name=f"w2_{c}")
            nc.gpsimd.dma_start(out=t[:], in_=w2b_d[c * 128:(c + 1) * 128, :])
            w2_t.append(t)
        b2_t = sb.tile([1, O], bf16, tag="b2")
        nc.gpsimd.dma_start(out=b2_t[:], in_=w2b_d[H:H + 1, :])

        # PE pstate warm-up on memset tiles (no DMA dependency); the real
        # accumulation below opens with start=True, discarding this junk
        wa = sb.tile([128, 128], bf16, tag="wa")
        nc.vector.memset(wa[:], 0.0)
        wb = sb.tile([128, E], bf16, tag="wb")
        nc.vector.memset(wb[:], 0.0)

        pooled_all = sb.tile([128, BG * E], bf16, tag="pooled_all")
        with tc.tile_pool(name="psA", bufs=1, space="PSUM") as psA:
            acc = [
                psA.tile([128, 512], f32, tag=f"acc{g}", name=f"acc{g}")
                for g in range(BG)
            ]
            for w in range(NWARM):
                nc.tensor.matmul(out=acc[0][:, 0:E], lhsT=wa[:], rhs=wb[:],
                                 start=True, stop=True)
            for rep in range(repeat):
                for k in range(KC):
                    ct = cnt_t[k // 2]
                    t = k % 2
                    for g in range(BG):
                        nc.tensor.matmul(
                            out=acc[g][:, 0:E],
                            lhsT=ct[:, t * B + g * 128:t * B + (g + 1) * 128],
                            rhs=emb_t[k][:],
                            start=(k == 0),
                            stop=(k == KC - 1),
                        )
            # drain the accumulators (pipelines behind the last matmuls;
            # gpsimd cannot read PSUM)
            for g in range(BG):
                nc.vector.tensor_copy(
                    out=pooled_all[:, g * E:(g + 1) * E], in_=acc[g][:, 0:E]
                )

        # cross-core sum + scatter: core i keeps batch rows [128i, 128i+128)
        part_d = dram.tile([B, E], bf16)
        rs_d = dram.tile([BS, E], bf16)
        nc.gpsimd.dma_start(
            out=part_d[:].rearrange("(g p) e -> p g e", g=BG),
            in_=pooled_all[:].rearrange("p (g e) -> p g e", g=BG),
        )
        nc.gpsimd.collective_compute(
            "ReduceScatter",
            mybir.AluOpType.add,
            replica_groups=[list(range(NCORES))],
            ins=[part_d.opt()],
            outs=[rs_d.opt()],
        )
        pooled = sb.tile([BS, E], bf16, tag="pooled")
        nc.gpsimd.dma_start(out=pooled[:], in_=rs_d[:])

        with tc.tile_pool(name="ps", bufs=1, space="PSUM") as ps, \
                tc.tile_pool(name="ps2", bufs=2, space="PSUM") as ps2:
            # fc1: h = relu(pooled @ W1 + b1), contraction via pooled^T on PE
            ident = sb.tile([128, 128], bf16, tag="ident")
            make_identity(nc, ident[:])
            lhs = []
            for c, (c0, c1) in enumerate([(0, 128), (128, 256), (256, E)]):
                w = c1 - c0
                pt = ps2.tile([w, 128], bf16, tag="tr", space="PSUM")
                nc.tensor.transpose(out=pt[:], in_=pooled[:, c0:c1],
                                    identity=ident[:])
                rows = w + 1 if c == 2 else w
                lt = sb.tile([rows, 128], bf16, tag=f"lhs{c}", name=f"lhs{c}")
                if c == 2:
                    # row `w` must be ones (bias row); memset whole tile first
                    # (partition-offset writes must start at partition 0)
                    nc.vector.memset(lt[:], 1.0)
                nc.vector.tensor_copy(out=lt[0:w, :], in_=pt[:])
                lhs.append(lt)
            hp = ps.tile([128, H], f32, tag="hp", space="PSUM")
            for c in range(3):
                nc.tensor.matmul(
                    out=hp[:], lhsT=lhs[c][:], rhs=w1_t[c][:],
                    start=(c == 0), stop=(c == 2),
                )
            h = sb.tile([128, H], bf16, tag="h")
            nc.scalar.activation(out=h[:], in_=hp[:],
                                 func=mybir.ActivationFunctionType.Relu)

            # fc2: out = h @ W2 + b2
            ones1 = sb.tile([1, 128], bf16, tag="ones1")
            nc.vector.memset(ones1[:], 1.0)
            op_ = ps.tile([128, O], f32, tag="op", space="PSUM")
            for c in range(4):
                pt = ps2.tile([128, 128], bf16, tag="tr2", space="PSUM")
                nc.tensor.transpose(out=pt[:], in_=h[:, c * 128:(c + 1) * 128],
                                    identity=ident[:])
                ht = sb.tile([128, 128], bf16, tag=f"ht{c}", name=f"ht{c}")
                nc.vector.tensor_copy(out=ht[:], in_=pt[:])
                nc.tensor.matmul(out=op_[:], lhsT=ht[:], rhs=w2_t[c][:],
                                 start=(c == 0), stop=False)
            nc.tensor.matmul(out=op_[:], lhsT=ones1[:], rhs=b2_t[:],
                             start=False, stop=True)
            out_sb = sb.tile([BS, O], f32, tag="osb")
            nc.vector.tensor_copy(out=out_sb[:], in_=op_[:])
            nc.sync.dma_start(out=out_d[:], in_=out_sb[:])

    nc.finalize()
    return nc


def _prep_in_maps(text, lengths, emb_table, W1, b1, W2, b2):
    import ml_dtypes

    bf16 = ml_dtypes.bfloat16
    text = np.asarray(text, dtype=np.int64)         # [S, B]
    lengths = np.asarray(lengths, dtype=np.int64)   # [B]

    # counts^T [VP, B] scaled by 1/len: row v = per-batch frequency of
    # token v among the first len[b] positions (vocab-major for sharding)
    mask = np.arange(S)[:, None] < lengths[None, :]
    flat = (text * B + np.arange(B)[None, :])[mask]
    cntT = np.bincount(flat, minlength=VP * B).reshape(VP, B)
    inv_len = (1.0 / lengths.astype(np.float32)).astype(np.float32)
    cntT16 = (cntT * inv_len[None, :]).astype(bf16)

    embp = np.zeros((VP, E), np.float32)
    embp[:V] = np.asarray(emb_table, np.float32)
    emb16 = embp.astype(bf16)

    w1b = np.vstack([np.asarray(W1, np.float32),
                     np.asarray(b1, np.float32)[None, :]]).astype(bf16)
    w2b = np.vstack([np.asarray(W2, np.float32),
                     np.asarray(b2, np.float32)[None, :]]).astype(bf16)

    in_maps = []
    for i in range(NCORES):
        in_maps.append({
            "cnt": np.ascontiguousarray(cntT16[i * VSH:(i + 1) * VSH]),
            "emb": np.ascontiguousarray(emb16[i * VSH:(i + 1) * VSH]),
            "w1b": w1b,
            "w2b": w2b,
        })
    return in_maps


def _run(inputs, trace=False):
    from concourse.bass_utils import run_bass_kernel_spmd

    nc = _build_nc()
    in_maps = _prep_in_maps(**inputs)
    res = run_bass_kernel_spmd(nc, in_maps, list(range(NCORES)), trace=trace)
    out = np.concatenate([res.results[i]["out"] for i in range(NCORES)], axis=0)
    return out.astype(np.float32), res


def kernel(**inputs):
    out, _ = _run(inputs, trace=False)
    return out
